# revision 1
# baseline (speedup 1.0000x reference)
"""AuxSeLoss v4: the v1 dataflow (ACT softplus + DVE dots/t-reduce in
parallel; PE only for tiny stat folds) with overhead trims:
  - F=5376 chunks: 26 ACTIVATE instead of 34, 13 accumulator reads
    instead of 17 (the reads cost 279ns each on the critical ACT queue).
  - dot outputs overwrite the spent e tiles, freeing SBUF for the big
    chunks (no separate garbage tile).
  - per-chunk stats fold into PSUM via a ones-matmul with start/stop, so
    the tail is one copy + DMA (no V collapse).
  - chunk-0 input DMA is triggered before the out2 warmup DMA, so the
    bulk stream starts ~1us earlier; the warmup still forces the single
    exp/ln ACT table load to happen early.
Lesson from v2/v3: adding GpSimd or PE bulk work oversubscribes the SBUF
ports and slows DVE/ACT/DMA below their solo rates -- so everything heavy
stays exactly where v1 had it.
"""

import numpy as np

N_CLASSES = 21
B, C, H, W = 16, N_CLASSES, 256, 256
N_CORES = 8
B_LOCAL = B // N_CORES
ELEMS_PER_SAMPLE = C * H * W  # 1376256
P = 128
FREE_PER_SAMPLE = ELEMS_PER_SAMPLE // P  # 10752
ROWS = B_LOCAL * P
AUX_WEIGHT = 0.4
SE_WEIGHT = 0.2
N_TOTAL = B * C * H * W
N_SE = B * C

CHUNK_SCHEDULE = [
    [672, 5376, 4704],  # sample 0: small first chunk -> fast ACT start
    [5376, 5040, 336],  # sample 1: tiny last chunk -> short tail
]
assert all(sum(cs) == FREE_PER_SAMPLE for cs in CHUNK_SCHEDULE)
FMAX = 5376
FALLOC = 5448  # pad tiles +288B so DMA/engine SBUF bank phases differ
NSTAT = 8  # Vc cols per sample half: 0=sp0 1=sp1 2=d0 3=d1 4=tsum 7=sp2

_CACHE: dict = {}


def _build():
    from contextlib import ExitStack

    import concourse.bacc as bacc
    import concourse.mybir as mybir
    from concourse.tile import TileContext

    f32 = mybir.dt.float32
    AFT = mybir.ActivationFunctionType
    ALU = mybir.AluOpType

    import concourse.hw_specs as hw_specs

    tables = hw_specs.get_activation_tables("gen3")
    combined = "natural_log_exp_and_others"
    if combined in tables and {AFT.Exp, AFT.Ln} <= tables[combined]:
        for name, funcs in tables.items():
            if name != combined:
                funcs.discard(AFT.Exp)
                funcs.discard(AFT.Ln)

    nc = bacc.Bacc("TRN2", target_bir_lowering=False)
    x0 = nc.dram_tensor("out0", [ROWS, FREE_PER_SAMPLE], f32, kind="ExternalInput")
    x1 = nc.dram_tensor("out1", [ROWS, FREE_PER_SAMPLE], f32, kind="ExternalInput")
    tg = nc.dram_tensor("targets", [ROWS, FREE_PER_SAMPLE], f32, kind="ExternalInput")
    o2 = nc.dram_tensor("out2", [1, B_LOCAL * C], f32, kind="ExternalInput")
    res = nc.dram_tensor("stats", [1, 16], f32, kind="ExternalOutput")

    with ExitStack() as ctx, TileContext(nc) as tc:
        with (
            tc.tile_pool(name="tp", bufs=2) as tp,
            tc.tile_pool(name="x0p", bufs=2) as x0p,
            tc.tile_pool(name="x1p", bufs=2) as x1p,
            tc.tile_pool(name="ep", bufs=2) as ep,
            tc.tile_pool(name="gdp", bufs=1) as gdp,
            tc.tile_pool(name="vcp", bufs=2) as vcp,
            tc.tile_pool(name="accp", bufs=1) as accp,
            tc.tile_pool(name="psp", bufs=1, space="PSUM") as psp,
        ):
            ones_t = accp.tile([P, 1], f32)
            nc.vector.memset(ones_t[:], 1.0)
            Us = accp.tile([1, 16], f32)
            o2_t = accp.tile([1, B_LOCAL * C], f32)
            e_o2 = accp.tile([1, B_LOCAL * C], f32)
            g_o2 = accp.tile([1, B_LOCAL * C], f32)
            U = psp.tile([1, 16], f32)

            chunks = []
            for s in range(B_LOCAL):
                c0 = 0
                for cols in CHUNK_SCHEDULE[s]:
                    chunks.append((s, c0, cols))
                    c0 += cols
            n_chunks = len(chunks)

            # x0 tiles are allocated/DMA'd one chunk ahead so the ACT
            # engine (whose first op per chunk reads x0) never waits:
            # issue order: x0(c0), then per chunk c: t(c), x1(c), x0(c+1).
            # Depth-1 keeps the x0(c+1) trigger's buffer-wait (on compute of
            # c-1) already satisfied when the sync engine reaches it.
            x0_tiles = []

            def x0_tile(ci):
                s, c0, cols = chunks[ci]
                r0, r1 = s * P, (s + 1) * P
                xt = x0p.tile([P, FALLOC], f32, name=f"x0_{ci}", tag="x0")
                nc.sync.dma_start(xt[:, 0:cols], x0[r0:r1, c0 : c0 + cols])
                x0_tiles.append(xt)
                return xt

            x0_tile(0)
            first = True
            for ci, (s, c0, cols) in enumerate(chunks):
                r0, r1 = s * P, (s + 1) * P
                c1 = c0 + cols
                t_t = tp.tile([P, FALLOC], f32, name=f"t_{ci}", tag="t")
                x0_t = x0_tiles[ci]
                x1_t = x1p.tile([P, FALLOC], f32, name=f"x1_{ci}", tag="x1")
                nc.sync.dma_start(t_t[:, 0:cols], tg[r0:r1, c0:c1])
                nc.sync.dma_start(x1_t[:, 0:cols], x1[r0:r1, c0:c1])
                if ci + 1 < len(chunks):
                    x0_tile(ci + 1)

                Vc = vcp.tile([P, 16], f32, name=f"vc_{ci}", tag="vc")
                nc.gpsimd.memset(Vc[:], 0.0)
                sc = s * NSTAT

                if first:
                    # out2 DMA + softplus warmup: forces the act table load
                    # early; issued after chunk-0's bulk DMA triggers.
                    nc.sync.dma_start(o2_t[:], o2[0:1, :])
                    nc.scalar.activation(e_o2[:], o2_t[:], AFT.Exp)
                    nc.scalar.activation(
                        g_o2[:], e_o2[:], AFT.Ln, bias=1.0,
                        accum_out=Vc[0:1, sc + 7 : sc + 8],
                    )
                    first = False

                e0 = ep.tile([P, FALLOC], f32, name=f"e0_{ci}", tag="e")
                e1 = ep.tile([P, FALLOC], f32, name=f"e1_{ci}", tag="e")
                # ACT: softplus via exp then ln(1+.) in place, sum fused
                # into the activation accumulator.
                nc.scalar.activation(e0[:, 0:cols], x0_t[:, 0:cols], AFT.Exp)
                nc.scalar.activation(
                    e0[:, 0:cols], e0[:, 0:cols], AFT.Ln, bias=1.0,
                    accum_out=Vc[:, sc + 0 : sc + 1],
                )
                nc.scalar.activation(e1[:, 0:cols], x1_t[:, 0:cols], AFT.Exp)
                nc.scalar.activation(
                    e1[:, 0:cols], e1[:, 0:cols], AFT.Ln, bias=1.0,
                    accum_out=Vc[:, sc + 1 : sc + 2],
                )

                # DVE: x.t dots into an independent scratch tile so they
                # run in parallel with ACT, plus the exact per-sample t sum.
                gd = gdp.tile([P, FALLOC], f32, name=f"gd_{ci}", tag="gd")
                nc.vector.scalar_tensor_tensor(
                    out=gd[:, 0:cols], in0=x0_t[:, 0:cols], scalar=1.0,
                    in1=t_t[:, 0:cols], op0=ALU.mult, op1=ALU.mult,
                    accum_out=Vc[:, sc + 2 : sc + 3],
                )
                nc.vector.scalar_tensor_tensor(
                    out=gd[:, 0:cols], in0=x1_t[:, 0:cols], scalar=1.0,
                    in1=t_t[:, 0:cols], op0=ALU.mult, op1=ALU.mult,
                    accum_out=Vc[:, sc + 3 : sc + 4],
                )
                nc.vector.tensor_reduce(
                    out=Vc[:, sc + 4 : sc + 5], in_=t_t[:, 0:cols],
                    axis=mybir.AxisListType.X, op=ALU.add,
                )

                # Fold this chunk's stats into PSUM (ones-matmul is exact
                # in fp32r for these integer-ish sums).
                nc.tensor.matmul(
                    U[:], ones_t[:], Vc[:],
                    start=(ci == 0), stop=(ci == n_chunks - 1),
                )

            nc.vector.tensor_copy(Us[:], U[:])
            nc.sync.dma_start(res[0:1, :], Us[:])

    nc.finalize()
    return nc


def _get_nc():
    if "nc" not in _CACHE:
        _CACHE["nc"] = _build()
    return _CACHE["nc"]


def _run(in_maps, trace=False):
    from concourse.bass_utils import run_bass_kernel_spmd

    return run_bass_kernel_spmd(
        _get_nc(), in_maps, core_ids=list(range(N_CORES)), trace=trace
    )


def make_in_maps(out0, out1, out2, targets):
    in_maps = []
    for c in range(N_CORES):
        sl = slice(c * B_LOCAL, (c + 1) * B_LOCAL)
        in_maps.append(
            {
                "out0": np.ascontiguousarray(out0[sl]).reshape(ROWS, FREE_PER_SAMPLE),
                "out1": np.ascontiguousarray(out1[sl]).reshape(ROWS, FREE_PER_SAMPLE),
                "targets": np.ascontiguousarray(targets[sl]).reshape(
                    ROWS, FREE_PER_SAMPLE
                ),
                "out2": np.ascontiguousarray(out2[sl]).reshape(1, B_LOCAL * C),
            }
        )
    return in_maps


def combine_partials(stats, out2):
    """Host-side O(1) combine. stats: [N_CORES, 16] per-core sums."""
    total_main = 0.0
    total_se = 0.0
    for c in range(len(stats)):
        v = [float(x) for x in stats[c]]
        total_se += v[7]  # sp2
        for s in range(B_LOCAL):
            o = s * NSTAT
            total_main += (v[o + 0] - v[o + 2]) + AUX_WEIGHT * (v[o + 1] - v[o + 3])
            t_sum = v[o + 4]
            b_global = c * B_LOCAL + s
            if t_sum < ELEMS_PER_SAMPLE - 0.5:  # class-bin 0 present
                total_se -= float(out2[b_global, 0])
            if t_sum > 0.5:  # class-bin 1 present
                total_se -= float(out2[b_global, 1])
    return total_main / N_TOTAL + SE_WEIGHT * total_se / N_SE


def kernel(out0, out1, out2, targets):
    out0 = np.asarray(out0, dtype=np.float32)
    out1 = np.asarray(out1, dtype=np.float32)
    out2 = np.asarray(out2, dtype=np.float32)
    targets = np.asarray(targets, dtype=np.float32)
    br = _run(make_in_maps(out0, out1, out2, targets))
    stats = [r["stats"][0] for r in br.results]
    return np.asarray(combine_partials(stats, out2), dtype=np.float32)



# revision 7
# speedup vs baseline: 1.0772x; 1.0772x over previous
"""AuxSeLoss v5: bf16 inputs + single-sigmoid softplus approximation.

Math: t in {0,1} exactly, so per-element BCE = softplus((1-2t)x) with
|(1-2t)x| = |x|.  Decompose softplus(z) = relu(z) + g(|z|) with
g(u) = log1p(exp(-u)), and approximate g(u) ~= A*sigmoid(-B*u+G) + D
(minimax fit, max |err| 4.9e-4 on u in [0,9]).  Then per tensor:
  sum BCE = 0.5*sum|x| - sum((t-0.5)*x) + A*sum sigmoid(-B|x|+G) + D*N
Engine mapping per chunk (all bulk tiles bf16):
  DVE: abs via tensor_scalar(abs_max,0) -> |x| tile (+accum sum|x|);
       stt (t-0.5)*x overwriting the spent x tile (+accum).
  ACT: one Sigmoid pass per tensor, in place on the |x| tile, sum fused
       into the activation accumulator.  2 ACT passes/chunk instead of
       the 4 exp/ln passes of v4 -> ACT ~36us, below the DMA floor.
  PE:  per-sample sum(t) via ones[128,1].T @ t 512-col blocks
       accumulated in PSUM (exact integer sums), plus the per-chunk
       [P,16] stat fold like v4.
  DMA: 3 tensors * 2.75M elems * 2B = 16.5 MB/core -> ~48us at the
       ~340GB/s/core HBM rate the v4 trace measured (f32 was 97us).
Host: cast inputs to bf16 (round-to-nearest via ml_dtypes), O(1)
combine of the 16 per-core sums.  End-to-end rel err ~1e-4 (gate 2e-2).
"""

import numpy as np
import ml_dtypes

N_CLASSES = 21
B, C, H, W = 16, N_CLASSES, 256, 256
N_CORES = 8
B_LOCAL = B // N_CORES
ELEMS_PER_SAMPLE = C * H * W  # 1376256
P = 128
FREE_PER_SAMPLE = ELEMS_PER_SAMPLE // P  # 10752
ROWS = B_LOCAL * P
AUX_WEIGHT = 0.4
SE_WEIGHT = 0.2
N_TOTAL = B * C * H * W
N_SE = B * C

# minimax fit of log1p(exp(-u)) ~= ALPHA*sigmoid(-BETA*u+GAMMA)+DELTA, u>=0
ALPHA = 2.49059269
BETA = 0.985901754
GAMMA = -0.954224925
DELTA = 1.79845165e-4

CHUNK_SCHEDULE = [
    [512, 5120, 5120],  # sample 0: small first chunk -> fast engine start
    [5120, 5120, 512],  # sample 1: tiny last chunk -> short tail
]
assert all(sum(cs) == FREE_PER_SAMPLE for cs in CHUNK_SCHEDULE)
assert all(c % 512 == 0 for cs in CHUNK_SCHEDULE for c in cs)
FMAX = 5120
FALLOC = 5264  # pad tiles +288B so DMA/engine SBUF bank phases differ
NSTAT = 8  # per-sample cols: 0=|x0| 1=z0 2=sig0 3=|x1| 4=z1 5=sig1 6=tsum
TBLK = 512  # PE t-sum block width (one PSUM bank of f32)

_CACHE: dict = {}


def _build():
    from contextlib import ExitStack

    import concourse.bacc as bacc
    import concourse.mybir as mybir
    from concourse.tile import TileContext

    f32 = mybir.dt.float32
    bf16 = mybir.dt.bfloat16
    AFT = mybir.ActivationFunctionType
    ALU = mybir.AluOpType

    nc = bacc.Bacc("TRN2", target_bir_lowering=False)
    x0 = nc.dram_tensor("out0", [ROWS, FREE_PER_SAMPLE], bf16, kind="ExternalInput")
    x1 = nc.dram_tensor("out1", [ROWS, FREE_PER_SAMPLE], bf16, kind="ExternalInput")
    tg = nc.dram_tensor("targets", [ROWS, FREE_PER_SAMPLE], bf16, kind="ExternalInput")
    o2 = nc.dram_tensor("out2", [1, B_LOCAL * C], f32, kind="ExternalInput")
    res = nc.dram_tensor("stats", [1, 16], f32, kind="ExternalOutput")

    with ExitStack() as ctx, TileContext(nc) as tc:
        with (
            tc.tile_pool(name="x0p", bufs=3) as x0p,
            tc.tile_pool(name="x1p", bufs=3) as x1p,
            tc.tile_pool(name="tp", bufs=3) as tp,
            tc.tile_pool(name="ap", bufs=4) as ap,
            tc.tile_pool(name="vcp", bufs=2) as vcp,
            tc.tile_pool(name="accp", bufs=1) as accp,
            tc.tile_pool(name="psp", bufs=1, space="PSUM") as psp,
        ):
            ones_f = accp.tile([P, 1], f32)
            nc.vector.memset(ones_f[:], 1.0)
            ones_b = accp.tile([P, 1], bf16)
            nc.vector.memset(ones_b[:], 1.0)
            gam_t = accp.tile([P, 1], f32)
            nc.vector.memset(gam_t[:], GAMMA)
            Us = accp.tile([1, 16], f32)
            o2_t = accp.tile([1, B_LOCAL * C], f32)
            a_o2 = accp.tile([1, B_LOCAL * C], f32)
            r_o2 = accp.tile([1, B_LOCAL * C], f32)
            U = psp.tile([1, 16], f32)
            pt = [psp.tile([1, TBLK], f32, name=f"pt{s}") for s in range(B_LOCAL)]

            chunks = []
            for s in range(B_LOCAL):
                c0 = 0
                for cols in CHUNK_SCHEDULE[s]:
                    chunks.append((s, c0, cols))
                    c0 += cols
            n_chunks = len(chunks)
            nblk = FREE_PER_SAMPLE // TBLK  # t-sum blocks per sample

            first = True
            for ci, (s, c0, cols) in enumerate(chunks):
                r0, r1 = s * P, (s + 1) * P
                c1 = c0 + cols
                x0_t = x0p.tile([P, FALLOC], bf16, name=f"x0_{ci}", tag="x0")
                t_t = tp.tile([P, FALLOC], bf16, name=f"t_{ci}", tag="t")
                x1_t = x1p.tile([P, FALLOC], bf16, name=f"x1_{ci}", tag="x1")
                nc.sync.dma_start(x0_t[:, 0:cols], x0[r0:r1, c0:c1])
                nc.sync.dma_start(t_t[:, 0:cols], tg[r0:r1, c0:c1])
                nc.sync.dma_start(x1_t[:, 0:cols], x1[r0:r1, c0:c1])

                Vc = vcp.tile([P, 16], f32, name=f"vc_{ci}", tag="vc")
                nc.gpsimd.memset(Vc[:], 0.0)
                sc = s * NSTAT

                if first:
                    # out2 warmup: tiny abs+sigmoid forces the single
                    # sigmoid table load before the first bulk ACT op.
                    nc.sync.dma_start(o2_t[:], o2[0:1, :])
                    nc.vector.scalar_tensor_tensor(
                        out=a_o2[:], in0=o2_t[:], scalar=-1.0, in1=o2_t[:],
                        op0=ALU.mult, op1=ALU.max,
                    )
                    # sum relu(o2) via tensor_scalar max(o2,0) -> col 7
                    nc.vector.tensor_scalar(
                        out=r_o2[:], in0=o2_t[:], scalar1=0.0, scalar2=None,
                        op0=ALU.max, op1=ALU.add,
                        accum_out=Vc[0:1, 7:8],
                    )
                    nc.scalar.activation(
                        a_o2[:], a_o2[:], AFT.Sigmoid,
                        bias=gam_t[0:1], scale=-BETA,
                        accum_out=Vc[0:1, 15:16],
                    )
                    first = False

                a0 = ap.tile([P, FALLOC], bf16, name=f"a0_{ci}", tag="a")
                a1 = ap.tile([P, FALLOC], bf16, name=f"a1_{ci}", tag="a")

                # DVE: |x| via stt (x abs_max 0) max x, then (t-0.5)*x over
                # the spent x tile, sums fused into the accumulator.
                nc.vector.scalar_tensor_tensor(
                    out=a0[:, 0:cols], in0=x0_t[:, 0:cols], scalar=-1.0,
                    in1=x0_t[:, 0:cols], op0=ALU.mult, op1=ALU.max,
                    accum_out=Vc[:, sc + 0 : sc + 1],
                )
                nc.vector.scalar_tensor_tensor(
                    out=x0_t[:, 0:cols], in0=t_t[:, 0:cols], scalar=0.5,
                    in1=x0_t[:, 0:cols], op0=ALU.subtract, op1=ALU.mult,
                    accum_out=Vc[:, sc + 1 : sc + 2],
                )
                nc.vector.scalar_tensor_tensor(
                    out=a1[:, 0:cols], in0=x1_t[:, 0:cols], scalar=-1.0,
                    in1=x1_t[:, 0:cols], op0=ALU.mult, op1=ALU.max,
                    accum_out=Vc[:, sc + 3 : sc + 4],
                )
                nc.vector.scalar_tensor_tensor(
                    out=x1_t[:, 0:cols], in0=t_t[:, 0:cols], scalar=0.5,
                    in1=x1_t[:, 0:cols], op0=ALU.subtract, op1=ALU.mult,
                    accum_out=Vc[:, sc + 4 : sc + 5],
                )

                # ACT: one sigmoid pass per tensor, in place, accum fused.
                nc.scalar.activation(
                    a0[:, 0:cols], a0[:, 0:cols], AFT.Sigmoid,
                    bias=gam_t[:], scale=-BETA,
                    accum_out=Vc[:, sc + 2 : sc + 3],
                )
                nc.scalar.activation(
                    a1[:, 0:cols], a1[:, 0:cols], AFT.Sigmoid,
                    bias=gam_t[:], scale=-BETA,
                    accum_out=Vc[:, sc + 5 : sc + 6],
                )

                # PE: per-sample sum(t) in 512-col blocks -> PSUM accum.
                for j in range(cols // TBLK):
                    bi = c0 // TBLK + j
                    nc.tensor.matmul(
                        pt[s][:], ones_b[:],
                        t_t[:, j * TBLK : (j + 1) * TBLK],
                        start=(bi == 0), stop=(bi == nblk - 1),
                    )

                # Fold this chunk's stats into PSUM (fp32r ones-matmul).
                nc.tensor.matmul(
                    U[:], ones_f[:], Vc[:],
                    start=(ci == 0), stop=(ci == n_chunks - 1),
                )

            nc.vector.tensor_copy(Us[:], U[:])
            for s in range(B_LOCAL):
                nc.vector.tensor_reduce(
                    out=Us[0:1, s * NSTAT + 6 : s * NSTAT + 7], in_=pt[s][:],
                    axis=mybir.AxisListType.X, op=ALU.add,
                )
            nc.sync.dma_start(res[0:1, :], Us[:])

    nc.finalize()
    return nc


def _get_nc():
    if "nc" not in _CACHE:
        _CACHE["nc"] = _build()
    return _CACHE["nc"]


def _run(in_maps, trace=False):
    from concourse.bass_utils import run_bass_kernel_spmd

    return run_bass_kernel_spmd(
        _get_nc(), in_maps, core_ids=list(range(N_CORES)), trace=trace
    )


def make_in_maps(out0, out1, out2, targets):
    bf = ml_dtypes.bfloat16
    out0 = np.asarray(out0, dtype=np.float32).astype(bf)
    out1 = np.asarray(out1, dtype=np.float32).astype(bf)
    targets = np.asarray(targets, dtype=np.float32).astype(bf)
    out2 = np.asarray(out2, dtype=np.float32)
    in_maps = []
    for c in range(N_CORES):
        sl = slice(c * B_LOCAL, (c + 1) * B_LOCAL)
        in_maps.append(
            {
                "out0": np.ascontiguousarray(out0[sl]).reshape(ROWS, FREE_PER_SAMPLE),
                "out1": np.ascontiguousarray(out1[sl]).reshape(ROWS, FREE_PER_SAMPLE),
                "targets": np.ascontiguousarray(targets[sl]).reshape(
                    ROWS, FREE_PER_SAMPLE
                ),
                "out2": np.ascontiguousarray(out2[sl]).reshape(1, B_LOCAL * C),
            }
        )
    return in_maps


def combine_partials(stats, out2):
    """Host-side O(1) combine. stats: [N_CORES, 16] per-core sums."""
    total_main = 0.0
    total_se = 0.0
    for c in range(len(stats)):
        v = [float(x) for x in stats[c]]
        # out2 partial: sum relu(o2) + ALPHA*sum(sig) + DELTA*count
        total_se += v[7] + ALPHA * v[15] + DELTA * (B_LOCAL * C)
        for s in range(B_LOCAL):
            o = s * NSTAT
            s0 = 0.5 * v[o + 0] - v[o + 1] + ALPHA * v[o + 2] + DELTA * ELEMS_PER_SAMPLE
            s1 = 0.5 * v[o + 3] - v[o + 4] + ALPHA * v[o + 5] + DELTA * ELEMS_PER_SAMPLE
            total_main += s0 + AUX_WEIGHT * s1
            t_sum = v[o + 6]
            b_global = c * B_LOCAL + s
            if t_sum < ELEMS_PER_SAMPLE - 0.5:  # class-bin 0 present
                total_se -= float(out2[b_global, 0])
            if t_sum > 0.5:  # class-bin 1 present
                total_se -= float(out2[b_global, 1])
    return total_main / N_TOTAL + SE_WEIGHT * total_se / N_SE


def kernel(out0, out1, out2, targets):
    out2 = np.asarray(out2, dtype=np.float32)
    br = _run(make_in_maps(out0, out1, out2, targets))
    stats = [r["stats"][0] for r in br.results]
    return np.asarray(combine_partials(stats, out2), dtype=np.float32)


# revision 8
# speedup vs baseline: 1.2671x; 1.1762x over previous
"""AuxSeLoss v6: bf16 inputs, mode-aware DVE dataflow, single-sigmoid ACT.

Math: t in {0,1} exactly, so per-element BCE = softplus(z), z = (1-2t)x.
softplus(z) = relu(z) + g(|z|), g(u) = log1p(exp(-u)) ~= A*sigmoid(-B*u+G)+D
(minimax fit, max |err| 4.9e-4).

DVE perf modes measured on HW: plain tensor_tensor = 2x, plain
tensor_scalar (incl 2-scalar-op form) = 4x, but ANY fused-accum form
(stt / ts+accum / custom reduce) = 1x.  So v6 uses no DVE accumulators:
  DVE per chunk (bf16):   s_t = t - 0.5            [ts, F/4]
    per tensor:           z'' = s_t * x (in place) [tt, F/2]  (z = -2z'')
                          rn  = (z''*-2) max 0     [ts 2-op, F/4] = relu(z)
                          a   = z'' + rn = |z''|   [tt add, F/2]
  ACT: one Sigmoid pass per tensor on a, scale=-2B bias=G, sum fused into
       the ACT accumulator (accum costs nothing on ACT).
  PE:  ones-matmul chains in 512-col blocks -> PSUM accumulation for
       sum(t) per sample (histogram bins) and sum(rn) per tensor
       (exact relu part); plus the per-chunk [P,16] Vc stat fold.
  DMA: 16.5 MB/core bf16 -> ~44.5us at the measured ~370GB/s.
Totals/core: DVE 2.75F ~= 62us (bound), ACT ~46us, PE ~40us, DMA ~45us.
Host: bf16 cast (ml_dtypes round-to-nearest), O(1) combine.
"""

import numpy as np
import ml_dtypes

N_CLASSES = 21
B, C, H, W = 16, N_CLASSES, 256, 256
N_CORES = 8
B_LOCAL = B // N_CORES
ELEMS_PER_SAMPLE = C * H * W  # 1376256
P = 128
FREE_PER_SAMPLE = ELEMS_PER_SAMPLE // P  # 10752
ROWS = B_LOCAL * P
AUX_WEIGHT = 0.4
SE_WEIGHT = 0.2
N_TOTAL = B * C * H * W
N_SE = B * C
N_CORE_T = B_LOCAL * ELEMS_PER_SAMPLE  # elems per tensor per core

# minimax fit of log1p(exp(-u)) ~= ALPHA*sigmoid(-BETA*u+GAMMA)+DELTA, u>=0
ALPHA = 2.49059269
BETA = 0.985901754
GAMMA = -0.954224925
DELTA = 1.79845165e-4

CHUNK_SCHEDULE = [
    [512, 5120, 5120],  # sample 0: small first chunk -> fast engine start
    [5120, 5120, 512],  # sample 1: tiny last chunk -> short tail
]
assert all(sum(cs) == FREE_PER_SAMPLE for cs in CHUNK_SCHEDULE)
assert all(c % 512 == 0 for cs in CHUNK_SCHEDULE for c in cs)
FMAX = 5120
FALLOC = 5264  # pad tiles +288B so DMA/engine SBUF bank phases differ
TBLK = 512  # PE chain block width (one PSUM bank of f32)

_CACHE: dict = {}


def _build():
    from contextlib import ExitStack

    import concourse.bacc as bacc
    import concourse.mybir as mybir
    from concourse.tile import TileContext

    f32 = mybir.dt.float32
    bf16 = mybir.dt.bfloat16
    AFT = mybir.ActivationFunctionType
    ALU = mybir.AluOpType

    nc = bacc.Bacc("TRN2", target_bir_lowering=False)
    x0 = nc.dram_tensor("out0", [ROWS, FREE_PER_SAMPLE], bf16, kind="ExternalInput")
    x1 = nc.dram_tensor("out1", [ROWS, FREE_PER_SAMPLE], bf16, kind="ExternalInput")
    tg = nc.dram_tensor("targets", [ROWS, FREE_PER_SAMPLE], bf16, kind="ExternalInput")
    o2 = nc.dram_tensor("out2", [1, B_LOCAL * C], f32, kind="ExternalInput")
    res = nc.dram_tensor("stats", [1, 16], f32, kind="ExternalOutput")

    with ExitStack() as ctx, TileContext(nc) as tc:
        with (
            tc.tile_pool(name="x0p", bufs=2) as x0p,
            tc.tile_pool(name="x1p", bufs=2) as x1p,
            tc.tile_pool(name="tp", bufs=2) as tp,
            tc.tile_pool(name="sp", bufs=2) as sp,
            tc.tile_pool(name="rnp", bufs=4) as rnp,
            tc.tile_pool(name="ap", bufs=4) as ap,
            tc.tile_pool(name="vcp", bufs=2) as vcp,
            tc.tile_pool(name="accp", bufs=1) as accp,
            tc.tile_pool(name="psp", bufs=1, space="PSUM") as psp,
        ):
            ones_f = accp.tile([P, 1], f32)
            nc.vector.memset(ones_f[:], 1.0)
            ones_b = accp.tile([P, 1], bf16)
            nc.vector.memset(ones_b[:], 1.0)
            gam_t = accp.tile([P, 1], f32)
            nc.vector.memset(gam_t[:], GAMMA)
            Us = accp.tile([1, 16], f32)
            o2_t = accp.tile([1, B_LOCAL * C], f32)
            a_o2 = accp.tile([1, B_LOCAL * C], f32)
            r_o2 = accp.tile([1, B_LOCAL * C], f32)
            U = psp.tile([1, 16], f32)
            pt = [psp.tile([1, TBLK], f32, name=f"pt{s}") for s in range(B_LOCAL)]
            pr = [psp.tile([1, TBLK], f32, name=f"pr{i}") for i in range(2)]

            chunks = []
            for s in range(B_LOCAL):
                c0 = 0
                for cols in CHUNK_SCHEDULE[s]:
                    chunks.append((s, c0, cols))
                    c0 += cols
            n_chunks = len(chunks)
            nblk_s = FREE_PER_SAMPLE // TBLK  # t-chain blocks per sample
            nblk_r = B_LOCAL * nblk_s  # rn-chain blocks per tensor
            rblk = [0, 0]  # rn-chain block counters

            first = True
            for ci, (s, c0, cols) in enumerate(chunks):
                r0, r1 = s * P, (s + 1) * P
                c1 = c0 + cols
                x0_t = x0p.tile([P, FALLOC], bf16, name=f"x0_{ci}", tag="x0")
                t_t = tp.tile([P, FALLOC], bf16, name=f"t_{ci}", tag="t")
                x1_t = x1p.tile([P, FALLOC], bf16, name=f"x1_{ci}", tag="x1")
                nc.sync.dma_start(x0_t[:, 0:cols], x0[r0:r1, c0:c1])
                nc.sync.dma_start(t_t[:, 0:cols], tg[r0:r1, c0:c1])
                nc.sync.dma_start(x1_t[:, 0:cols], x1[r0:r1, c0:c1])

                Vc = vcp.tile([P, 16], f32, name=f"vc_{ci}", tag="vc")
                nc.gpsimd.memset(Vc[:], 0.0)

                if first:
                    # out2 warmup: tiny abs+sigmoid forces the single
                    # sigmoid table load before the first bulk ACT op.
                    nc.sync.dma_start(o2_t[:], o2[0:1, :])
                    nc.vector.scalar_tensor_tensor(
                        out=a_o2[:], in0=o2_t[:], scalar=-1.0, in1=o2_t[:],
                        op0=ALU.mult, op1=ALU.max,
                    )
                    # sum relu(o2) -> col 2 (tiny 1x ts+accum is fine here)
                    nc.vector.tensor_scalar(
                        out=r_o2[:], in0=o2_t[:], scalar1=0.0, scalar2=None,
                        op0=ALU.max, op1=ALU.add,
                        accum_out=Vc[0:1, 2:3],
                    )
                    nc.scalar.activation(
                        a_o2[:], a_o2[:], AFT.Sigmoid,
                        bias=gam_t[0:1], scale=-BETA,
                        accum_out=Vc[0:1, 3:4],
                    )
                    first = False

                # DVE (no accums -> fast modes): s_t, then per tensor
                # z''=s_t*x in place, rn=relu(z), a=|z''|.
                s_t = sp.tile([P, FALLOC], bf16, name=f"s_{ci}", tag="s")
                nc.vector.tensor_scalar(
                    out=s_t[:, 0:cols], in0=t_t[:, 0:cols], scalar1=0.5,
                    scalar2=None, op0=ALU.subtract,
                )
                rn = [None, None]
                for i, xt in enumerate((x0_t, x1_t)):
                    nc.vector.tensor_tensor(
                        xt[:, 0:cols], s_t[:, 0:cols], xt[:, 0:cols], ALU.mult
                    )
                    rn_t = rnp.tile([P, FALLOC], bf16, name=f"rn{i}_{ci}", tag="rn")
                    nc.vector.tensor_scalar(
                        out=rn_t[:, 0:cols], in0=xt[:, 0:cols], scalar1=-2.0,
                        scalar2=0.0, op0=ALU.mult, op1=ALU.max,
                    )
                    a_t = ap.tile([P, FALLOC], bf16, name=f"a{i}_{ci}", tag="a")
                    nc.vector.tensor_tensor(
                        a_t[:, 0:cols], xt[:, 0:cols], rn_t[:, 0:cols], ALU.add
                    )
                    # ACT: sigmoid on |z''| (=|x|/2), sum fused into the
                    # activation accumulator -> Vc col i.
                    nc.scalar.activation(
                        a_t[:, 0:cols], a_t[:, 0:cols], AFT.Sigmoid,
                        bias=gam_t[:], scale=-2.0 * BETA,
                        accum_out=Vc[:, i : i + 1],
                    )
                    rn[i] = rn_t

                # PE chains: sum(t) per sample, sum(relu(z)) per tensor.
                for j in range(cols // TBLK):
                    bi = c0 // TBLK + j
                    nc.tensor.matmul(
                        pt[s][:], ones_b[:],
                        t_t[:, j * TBLK : (j + 1) * TBLK],
                        start=(bi == 0), stop=(bi == nblk_s - 1),
                    )
                for i in range(2):
                    for j in range(cols // TBLK):
                        nc.tensor.matmul(
                            pr[i][:], ones_b[:],
                            rn[i][:, j * TBLK : (j + 1) * TBLK],
                            start=(rblk[i] == 0), stop=(rblk[i] == nblk_r - 1),
                        )
                        rblk[i] += 1

                # Fold this chunk's stats into PSUM (fp32r ones-matmul).
                nc.tensor.matmul(
                    U[:], ones_f[:], Vc[:],
                    start=(ci == 0), stop=(ci == n_chunks - 1),
                )

            nc.vector.tensor_copy(Us[:], U[:])
            for s in range(B_LOCAL):
                nc.vector.tensor_reduce(
                    out=Us[0:1, 4 + s : 5 + s], in_=pt[s][:],
                    axis=mybir.AxisListType.X, op=ALU.add,
                )
            for i in range(2):
                nc.vector.tensor_reduce(
                    out=Us[0:1, 6 + i : 7 + i], in_=pr[i][:],
                    axis=mybir.AxisListType.X, op=ALU.add,
                )
            nc.sync.dma_start(res[0:1, :], Us[:])

    nc.finalize()
    return nc


def _get_nc():
    if "nc" not in _CACHE:
        _CACHE["nc"] = _build()
    return _CACHE["nc"]


def _run(in_maps, trace=False):
    from concourse.bass_utils import run_bass_kernel_spmd

    return run_bass_kernel_spmd(
        _get_nc(), in_maps, core_ids=list(range(N_CORES)), trace=trace
    )


def make_in_maps(out0, out1, out2, targets):
    bf = ml_dtypes.bfloat16
    out0 = np.asarray(out0, dtype=np.float32).astype(bf)
    out1 = np.asarray(out1, dtype=np.float32).astype(bf)
    targets = np.asarray(targets, dtype=np.float32).astype(bf)
    out2 = np.asarray(out2, dtype=np.float32)
    in_maps = []
    for c in range(N_CORES):
        sl = slice(c * B_LOCAL, (c + 1) * B_LOCAL)
        in_maps.append(
            {
                "out0": np.ascontiguousarray(out0[sl]).reshape(ROWS, FREE_PER_SAMPLE),
                "out1": np.ascontiguousarray(out1[sl]).reshape(ROWS, FREE_PER_SAMPLE),
                "targets": np.ascontiguousarray(targets[sl]).reshape(
                    ROWS, FREE_PER_SAMPLE
                ),
                "out2": np.ascontiguousarray(out2[sl]).reshape(1, B_LOCAL * C),
            }
        )
    return in_maps


def combine_partials(stats, out2):
    """Host-side O(1) combine. stats: [N_CORES, 16] per-core sums.
    cols: 0=sum sig(x0) 1=sum sig(x1) 2=sum relu(o2) 3=sum sig(o2)
          4,5=sum(t) per sample  6,7=sum relu(z) per tensor."""
    total_main = 0.0
    total_se = 0.0
    for c in range(len(stats)):
        v = [float(x) for x in stats[c]]
        s0 = v[6] + ALPHA * v[0] + DELTA * N_CORE_T
        s1 = v[7] + ALPHA * v[1] + DELTA * N_CORE_T
        total_main += s0 + AUX_WEIGHT * s1
        total_se += v[2] + ALPHA * v[3] + DELTA * (B_LOCAL * C)
        for s in range(B_LOCAL):
            t_sum = v[4 + s]
            b_global = c * B_LOCAL + s
            if t_sum < ELEMS_PER_SAMPLE - 0.5:  # class-bin 0 present
                total_se -= float(out2[b_global, 0])
            if t_sum > 0.5:  # class-bin 1 present
                total_se -= float(out2[b_global, 1])
    return total_main / N_TOTAL + SE_WEIGHT * total_se / N_SE


def kernel(out0, out1, out2, targets):
    out2 = np.asarray(out2, dtype=np.float32)
    br = _run(make_in_maps(out0, out1, out2, targets))
    stats = [r["stats"][0] for r in br.results]
    return np.asarray(combine_partials(stats, out2), dtype=np.float32)


# revision 12
# speedup vs baseline: 1.2914x; 1.0192x over previous
"""AuxSeLoss v7: bf16, engine-balanced softplus via sigmoid+relu split.

Math: t in {0,1} exactly -> per-element BCE = softplus(z), z = (1-2t)x.
softplus(z) = relu(z) + g(|z|), g(u) = log1p(exp(-u)) ~= A*sigmoid(-B*u+G)+D
(minimax fit, max |err| 4.9e-4).  With z'' = (t-0.5)x (so z = -2z''):
relu(z) = rn := max(-2z'', 0), |z''| = z'' + rn.

Engine balance (HW-measured modes: TT=2x, plain TS=4x, any DVE accum=1x,
ACT=1/cyc with free exact accumulator, PE chain mm ~400ns/512 cols):
  DVE per chunk: s_t = t-0.5 [ts F/4]; per tensor: z''=s_t*x in place
    [tt F/2], rn on the back (1-rho) cols [ts 2-op], a=z''+rn in place
    [tt F/2], sigma in place [ACT].
  ACT: per tensor one full-width Sigmoid pass (scale=-2B, accum) plus a
    Relu pass (scale=-2, accum) on the front rho~1/3 cols - writing the
    same rn tile slice DVE skips; its accumulator supplies that slice's
    sum(relu(z)) exactly.
  PE: ones-chains (512 blocks) for per-sample sum(t) and the back-cols
    sum(rn); per-chunk [P,16] Vc stat fold.
  DMA: t rides the ACT HWDGE queue (nc.scalar), x0/x1 the sync queue, so
    the t tile for chunk c+1 never waits behind x1 of chunk c.
Host: bf16 cast + O(1) combine.  rel err ~1e-4 (gate 2e-2).
"""

import numpy as np
import ml_dtypes

N_CLASSES = 21
B, C, H, W = 16, N_CLASSES, 256, 256
N_CORES = 8
B_LOCAL = B // N_CORES
ELEMS_PER_SAMPLE = C * H * W  # 1376256
P = 128
FREE_PER_SAMPLE = ELEMS_PER_SAMPLE // P  # 10752
ROWS = B_LOCAL * P
AUX_WEIGHT = 0.4
SE_WEIGHT = 0.2
N_TOTAL = B * C * H * W
N_SE = B * C
N_CORE_T = B_LOCAL * ELEMS_PER_SAMPLE  # elems per tensor per core

# minimax fit of log1p(exp(-u)) ~= ALPHA*sigmoid(-BETA*u+GAMMA)+DELTA, u>=0
ALPHA = 2.49059269
BETA = 0.985901754
GAMMA = -0.954224925
DELTA = 1.79845165e-4

# (cols, act_relu_cols) per chunk; act cols ride ACT, rest DVE+PE.
CHUNK_SCHEDULE = [
    [(1024, 512), (5120, 1536), (4608, 1536)],
    [(5632, 2048), (5120, 1536)],
]
assert all(sum(c for c, _ in cs) == FREE_PER_SAMPLE for cs in CHUNK_SCHEDULE)
assert all(c % 512 == 0 and k % 512 == 0 and k < c
           for cs in CHUNK_SCHEDULE for c, k in cs)
FALLOC = 5776  # 5632 max cols + 288B pad so bank phases differ
TBLK = 512  # PE chain block width (one PSUM bank of f32)

_CACHE: dict = {}


def _build():
    from contextlib import ExitStack

    import concourse.bacc as bacc
    import concourse.mybir as mybir
    from concourse.tile import TileContext

    f32 = mybir.dt.float32
    bf16 = mybir.dt.bfloat16
    AFT = mybir.ActivationFunctionType
    ALU = mybir.AluOpType

    nc = bacc.Bacc("TRN2", target_bir_lowering=False)
    x0 = nc.dram_tensor("out0", [ROWS, FREE_PER_SAMPLE], bf16, kind="ExternalInput")
    x1 = nc.dram_tensor("out1", [ROWS, FREE_PER_SAMPLE], bf16, kind="ExternalInput")
    tg = nc.dram_tensor("targets", [ROWS, FREE_PER_SAMPLE], bf16, kind="ExternalInput")
    o2 = nc.dram_tensor("out2", [1, B_LOCAL * C], f32, kind="ExternalInput")
    res = nc.dram_tensor("stats", [1, 16], f32, kind="ExternalOutput")

    with ExitStack() as ctx, TileContext(nc) as tc:
        with (
            tc.tile_pool(name="x0p", bufs=3) as x0p,
            tc.tile_pool(name="x1p", bufs=3) as x1p,
            tc.tile_pool(name="tp", bufs=3) as tp,
            tc.tile_pool(name="sp", bufs=2) as sp,
            tc.tile_pool(name="rnp", bufs=4) as rnp,
            tc.tile_pool(name="vcp", bufs=2) as vcp,
            tc.tile_pool(name="accp", bufs=1) as accp,
            tc.tile_pool(name="psp", bufs=1, space="PSUM") as psp,
        ):
            ones_f = accp.tile([P, 1], f32)
            nc.vector.memset(ones_f[:], 1.0)
            ones_b = accp.tile([P, 1], bf16)
            nc.vector.memset(ones_b[:], 1.0)
            gam_t = accp.tile([P, 1], f32)
            nc.vector.memset(gam_t[:], GAMMA)
            Us = accp.tile([1, 16], f32)
            o2_t = accp.tile([1, B_LOCAL * C], f32)
            a_o2 = accp.tile([1, B_LOCAL * C], f32)
            r_o2 = accp.tile([1, B_LOCAL * C], f32)
            U = psp.tile([1, 16], f32)
            pt = [psp.tile([1, TBLK], f32, name=f"pt{s}") for s in range(B_LOCAL)]
            pr = [psp.tile([1, TBLK], f32, name=f"pr{i}") for i in range(2)]

            chunks = []
            for s in range(B_LOCAL):
                c0 = 0
                for cols, ka in CHUNK_SCHEDULE[s]:
                    chunks.append((s, c0, cols, ka))
                    c0 += cols
            n_chunks = len(chunks)
            # total PE rn-chain cols per tensor (back cols of every chunk)
            rtot = sum(c - k for cs in CHUNK_SCHEDULE for c, k in cs)
            rcols = [0, 0]

            first = True
            for ci, (s, c0, cols, ka) in enumerate(chunks):
                r0, r1 = s * P, (s + 1) * P
                c1 = c0 + cols
                t_t = tp.tile([P, FALLOC], bf16, name=f"t_{ci}", tag="t")
                x0_t = x0p.tile([P, FALLOC], bf16, name=f"x0_{ci}", tag="x0")
                x1_t = x1p.tile([P, FALLOC], bf16, name=f"x1_{ci}", tag="x1")
                # t on the ACT HWDGE queue; x0/x1 on the sync queue.
                nc.scalar.dma_start(t_t[:, 0:cols], tg[r0:r1, c0:c1])
                nc.sync.dma_start(x0_t[:, 0:cols], x0[r0:r1, c0:c1])
                nc.sync.dma_start(x1_t[:, 0:cols], x1[r0:r1, c0:c1])

                Vc = vcp.tile([P, 16], f32, name=f"vc_{ci}", tag="vc")
                nc.gpsimd.memset(Vc[:], 0.0)

                if first:
                    # out2 warmup: tiny abs+sigmoid forces the sigmoid
                    # table load before the first bulk ACT op.
                    nc.sync.dma_start(o2_t[:], o2[0:1, :])
                    nc.vector.scalar_tensor_tensor(
                        out=a_o2[:], in0=o2_t[:], scalar=-1.0, in1=o2_t[:],
                        op0=ALU.mult, op1=ALU.max,
                    )
                    nc.vector.tensor_scalar(
                        out=r_o2[:], in0=o2_t[:], scalar1=0.0, scalar2=None,
                        op0=ALU.max, op1=ALU.add,
                        accum_out=Vc[0:1, 4:5],
                    )
                    nc.scalar.activation(
                        a_o2[:], a_o2[:], AFT.Sigmoid,
                        bias=gam_t[0:1], scale=-BETA,
                        accum_out=Vc[0:1, 5:6],
                    )
                    first = False

                s_t = sp.tile([P, FALLOC], bf16, name=f"s_{ci}", tag="s")
                nc.vector.tensor_scalar(
                    out=s_t[:, 0:cols], in0=t_t[:, 0:cols], scalar1=0.5,
                    scalar2=None, op0=ALU.subtract,
                )
                for i, xt in enumerate((x0_t, x1_t)):
                    # z'' = s_t * x, in place over x
                    nc.vector.tensor_tensor(
                        xt[:, 0:cols], s_t[:, 0:cols], xt[:, 0:cols], ALU.mult
                    )
                    rn_t = rnp.tile([P, FALLOC], bf16, name=f"rn{i}_{ci}", tag="rn")
                    # front ka cols: ACT computes rn = relu(-2 z'') with an
                    # exact fused sum; back cols: DVE 2-op ts + PE chain.
                    nc.scalar.activation(
                        rn_t[:, 0:ka], xt[:, 0:ka], AFT.Relu,
                        scale=-2.0, accum_out=Vc[:, 2 + i : 3 + i],
                    )
                    nc.vector.tensor_scalar(
                        out=rn_t[:, ka:cols], in0=xt[:, ka:cols], scalar1=-2.0,
                        scalar2=0.0, op0=ALU.mult, op1=ALU.max,
                    )
                    # a = z'' + rn = |z''|, in place over x
                    nc.vector.tensor_tensor(
                        xt[:, 0:cols], xt[:, 0:cols], rn_t[:, 0:cols], ALU.add
                    )
                    # sigma pass, in place, exact fused sum
                    nc.scalar.activation(
                        xt[:, 0:cols], xt[:, 0:cols], AFT.Sigmoid,
                        bias=gam_t[:], scale=-2.0 * BETA,
                        accum_out=Vc[:, i : i + 1],
                    )
                    # PE chain over the back rn cols
                    for j in range(ka, cols, TBLK):
                        nc.tensor.matmul(
                            pr[i][:], ones_b[:], rn_t[:, j : j + TBLK],
                            start=(rcols[i] == 0),
                            stop=(rcols[i] + TBLK == rtot),
                        )
                        rcols[i] += TBLK

                # PE chain: per-sample sum(t)
                for j in range(cols // TBLK):
                    off = c0 + j * TBLK
                    nc.tensor.matmul(
                        pt[s][:], ones_b[:], t_t[:, j * TBLK : (j + 1) * TBLK],
                        start=(off == 0), stop=(off + TBLK == FREE_PER_SAMPLE),
                    )

                # fold this chunk's stats into PSUM (fp32r ones-matmul)
                nc.tensor.matmul(
                    U[:], ones_f[:], Vc[:],
                    start=(ci == 0), stop=(ci == n_chunks - 1),
                )

                if ci == len(CHUNK_SCHEDULE[0]) - 1:
                    # sample 0's t-chain just stopped; drain it early
                    nc.vector.tensor_reduce(
                        out=Us[0:1, 8:9], in_=pt[0][:],
                        axis=mybir.AxisListType.X, op=ALU.add,
                    )

            nc.vector.tensor_reduce(
                out=Us[0:1, 9:10], in_=pt[1][:],
                axis=mybir.AxisListType.X, op=ALU.add,
            )
            for i in range(2):
                nc.vector.tensor_reduce(
                    out=Us[0:1, 10 + i : 11 + i], in_=pr[i][:],
                    axis=mybir.AxisListType.X, op=ALU.add,
                )
            nc.vector.tensor_copy(Us[0:1, 0:8], U[0:1, 0:8])
            nc.sync.dma_start(res[0:1, :], Us[:])

    nc.finalize()
    return nc


def _get_nc():
    if "nc" not in _CACHE:
        _CACHE["nc"] = _build()
    return _CACHE["nc"]


def _run(in_maps, trace=False):
    from concourse.bass_utils import run_bass_kernel_spmd

    return run_bass_kernel_spmd(
        _get_nc(), in_maps, core_ids=list(range(N_CORES)), trace=trace
    )


def make_in_maps(out0, out1, out2, targets):
    bf = ml_dtypes.bfloat16
    out0 = np.asarray(out0, dtype=np.float32).astype(bf)
    out1 = np.asarray(out1, dtype=np.float32).astype(bf)
    targets = np.asarray(targets, dtype=np.float32).astype(bf)
    out2 = np.asarray(out2, dtype=np.float32)
    in_maps = []
    for c in range(N_CORES):
        sl = slice(c * B_LOCAL, (c + 1) * B_LOCAL)
        in_maps.append(
            {
                "out0": np.ascontiguousarray(out0[sl]).reshape(ROWS, FREE_PER_SAMPLE),
                "out1": np.ascontiguousarray(out1[sl]).reshape(ROWS, FREE_PER_SAMPLE),
                "targets": np.ascontiguousarray(targets[sl]).reshape(
                    ROWS, FREE_PER_SAMPLE
                ),
                "out2": np.ascontiguousarray(out2[sl]).reshape(1, B_LOCAL * C),
            }
        )
    return in_maps


def combine_partials(stats, out2):
    """Host-side O(1) combine. stats: [N_CORES, 16] per-core sums.
    cols: 0,1=sum sig per tensor; 2,3=ACT-side sum relu(z) per tensor;
    4=sum relu(o2); 5=sum sig(o2); 8,9=sum(t) per sample;
    10,11=PE-side sum relu(z) per tensor."""
    total_main = 0.0
    total_se = 0.0
    for c in range(len(stats)):
        v = [float(x) for x in stats[c]]
        s0 = v[2] + v[10] + ALPHA * v[0] + DELTA * N_CORE_T
        s1 = v[3] + v[11] + ALPHA * v[1] + DELTA * N_CORE_T
        total_main += s0 + AUX_WEIGHT * s1
        total_se += v[4] + ALPHA * v[5] + DELTA * (B_LOCAL * C)
        for s in range(B_LOCAL):
            t_sum = v[8 + s]
            b_global = c * B_LOCAL + s
            if t_sum < ELEMS_PER_SAMPLE - 0.5:  # class-bin 0 present
                total_se -= float(out2[b_global, 0])
            if t_sum > 0.5:  # class-bin 1 present
                total_se -= float(out2[b_global, 1])
    return total_main / N_TOTAL + SE_WEIGHT * total_se / N_SE


def kernel(out0, out1, out2, targets):
    out2 = np.asarray(out2, dtype=np.float32)
    br = _run(make_in_maps(out0, out1, out2, targets))
    stats = [r["stats"][0] for r in br.results]
    return np.asarray(combine_partials(stats, out2), dtype=np.float32)


# revision 13
# speedup vs baseline: 1.3587x; 1.0521x over previous
"""AuxSeLoss v7: bf16, engine-balanced softplus via sigmoid+relu split.

Math: t in {0,1} exactly -> per-element BCE = softplus(z), z = (1-2t)x.
softplus(z) = relu(z) + g(|z|), g(u) = log1p(exp(-u)) ~= A*sigmoid(-B*u+G)+D
(minimax fit, max |err| 4.9e-4).  With z'' = (t-0.5)x (so z = -2z''):
relu(z) = rn := max(-2z'', 0), |z''| = z'' + rn.

Engine balance (HW-measured modes: TT=2x, plain TS=4x, any DVE accum=1x,
ACT=1/cyc with free exact accumulator, PE chain mm ~400ns/512 cols):
  DVE per chunk: s_t = t-0.5 [ts F/4]; per tensor: z''=s_t*x in place
    [tt F/2], rn on the back (1-rho) cols [ts 2-op], a=z''+rn in place
    [tt F/2], sigma in place [ACT].
  ACT: per tensor one full-width Sigmoid pass (scale=-2B, accum) plus a
    Relu pass (scale=-2, accum) on the front rho~1/3 cols - writing the
    same rn tile slice DVE skips; its accumulator supplies that slice's
    sum(relu(z)) exactly.
  PE: ones-chains (512 blocks) for per-sample sum(t) and the back-cols
    sum(rn); per-chunk [P,16] Vc stat fold.
  DMA: t rides the ACT HWDGE queue (nc.scalar), x0/x1 the sync queue, so
    the t tile for chunk c+1 never waits behind x1 of chunk c.
Host: bf16 cast + O(1) combine.  rel err ~1e-4 (gate 2e-2).
"""

import numpy as np
import ml_dtypes

N_CLASSES = 21
B, C, H, W = 16, N_CLASSES, 256, 256
N_CORES = 8
B_LOCAL = B // N_CORES
ELEMS_PER_SAMPLE = C * H * W  # 1376256
P = 128
FREE_PER_SAMPLE = ELEMS_PER_SAMPLE // P  # 10752
ROWS = B_LOCAL * P
AUX_WEIGHT = 0.4
SE_WEIGHT = 0.2
N_TOTAL = B * C * H * W
N_SE = B * C
N_CORE_T = B_LOCAL * ELEMS_PER_SAMPLE  # elems per tensor per core

# minimax fit of log1p(exp(-u)) ~= ALPHA*sigmoid(-BETA*u+GAMMA)+DELTA, u>=0
ALPHA = 2.49059269
BETA = 0.985901754
GAMMA = -0.954224925
DELTA = 1.79845165e-4

# (cols, act_relu_cols) per chunk; act cols ride ACT, rest DVE+PE.
CHUNK_SCHEDULE = [
    [(1024, 0), (2048, 512), (3072, 1024), (4608, 1536)],
    [(5632, 2048), (4608, 1536), (512, 0)],
]
assert all(sum(c for c, _ in cs) == FREE_PER_SAMPLE for cs in CHUNK_SCHEDULE)
assert all(c % 512 == 0 and k % 512 == 0 and k < c
           for cs in CHUNK_SCHEDULE for c, k in cs)
N_CHUNKS0 = len(CHUNK_SCHEDULE[0])
FALLOC = 5776  # 5632 max cols + 288B pad so bank phases differ
TBLK = 512  # PE chain block width (one PSUM bank of f32)

_CACHE: dict = {}


def _build():
    from contextlib import ExitStack

    import concourse.bacc as bacc
    import concourse.mybir as mybir
    from concourse.tile import TileContext

    f32 = mybir.dt.float32
    bf16 = mybir.dt.bfloat16
    AFT = mybir.ActivationFunctionType
    ALU = mybir.AluOpType

    nc = bacc.Bacc("TRN2", target_bir_lowering=False)
    x0 = nc.dram_tensor("out0", [ROWS, FREE_PER_SAMPLE], bf16, kind="ExternalInput")
    x1 = nc.dram_tensor("out1", [ROWS, FREE_PER_SAMPLE], bf16, kind="ExternalInput")
    tg = nc.dram_tensor("targets", [ROWS, FREE_PER_SAMPLE], bf16, kind="ExternalInput")
    o2 = nc.dram_tensor("out2", [1, B_LOCAL * C], f32, kind="ExternalInput")
    res = nc.dram_tensor("stats", [1, 16], f32, kind="ExternalOutput")

    with ExitStack() as ctx, TileContext(nc) as tc:
        with (
            tc.tile_pool(name="x0p", bufs=3) as x0p,
            tc.tile_pool(name="x1p", bufs=3) as x1p,
            tc.tile_pool(name="tp", bufs=3) as tp,
            tc.tile_pool(name="sp", bufs=2) as sp,
            tc.tile_pool(name="rnp", bufs=4) as rnp,
            tc.tile_pool(name="vcp", bufs=2) as vcp,
            tc.tile_pool(name="accp", bufs=1) as accp,
            tc.tile_pool(name="psp", bufs=1, space="PSUM") as psp,
        ):
            ones_f = accp.tile([P, 1], f32)
            nc.vector.memset(ones_f[:], 1.0)
            ones_b = accp.tile([P, 1], bf16)
            nc.vector.memset(ones_b[:], 1.0)
            gam_t = accp.tile([P, 1], f32)
            nc.vector.memset(gam_t[:], GAMMA)
            Us = accp.tile([1, 16], f32)
            o2_t = accp.tile([1, B_LOCAL * C], f32)
            a_o2 = accp.tile([1, B_LOCAL * C], f32)
            r_o2 = accp.tile([1, B_LOCAL * C], f32)
            U = psp.tile([1, 16], f32)
            pt = [psp.tile([1, TBLK], f32, name=f"pt{s}") for s in range(B_LOCAL)]
            pr = [psp.tile([1, TBLK], f32, name=f"pr{i}") for i in range(2)]

            chunks = []
            for s in range(B_LOCAL):
                c0 = 0
                for cols, ka in CHUNK_SCHEDULE[s]:
                    chunks.append((s, c0, cols, ka))
                    c0 += cols
            n_chunks = len(chunks)
            # total PE rn-chain cols per tensor (back cols of every chunk)
            rtot = sum(c - k for cs in CHUNK_SCHEDULE for c, k in cs)
            rcols = [0, 0]

            first = True
            for ci, (s, c0, cols, ka) in enumerate(chunks):
                r0, r1 = s * P, (s + 1) * P
                c1 = c0 + cols
                t_t = tp.tile([P, FALLOC], bf16, name=f"t_{ci}", tag="t")
                x0_t = x0p.tile([P, FALLOC], bf16, name=f"x0_{ci}", tag="x0")
                x1_t = x1p.tile([P, FALLOC], bf16, name=f"x1_{ci}", tag="x1")
                nc.sync.dma_start(t_t[:, 0:cols], tg[r0:r1, c0:c1])
                nc.sync.dma_start(x0_t[:, 0:cols], x0[r0:r1, c0:c1])
                nc.sync.dma_start(x1_t[:, 0:cols], x1[r0:r1, c0:c1])

                Vc = vcp.tile([P, 16], f32, name=f"vc_{ci}", tag="vc")
                nc.gpsimd.memset(Vc[:], 0.0)

                if first:
                    # out2 warmup: tiny abs+sigmoid forces the sigmoid
                    # table load before the first bulk ACT op.
                    nc.sync.dma_start(o2_t[:], o2[0:1, :])
                    nc.vector.scalar_tensor_tensor(
                        out=a_o2[:], in0=o2_t[:], scalar=-1.0, in1=o2_t[:],
                        op0=ALU.mult, op1=ALU.max,
                    )
                    nc.vector.tensor_scalar(
                        out=r_o2[:], in0=o2_t[:], scalar1=0.0, scalar2=None,
                        op0=ALU.max, op1=ALU.add,
                        accum_out=Vc[0:1, 4:5],
                    )
                    nc.scalar.activation(
                        a_o2[:], a_o2[:], AFT.Sigmoid,
                        bias=gam_t[0:1], scale=-BETA,
                        accum_out=Vc[0:1, 5:6],
                    )
                    first = False

                s_t = sp.tile([P, FALLOC], bf16, name=f"s_{ci}", tag="s")
                nc.vector.tensor_scalar(
                    out=s_t[:, 0:cols], in0=t_t[:, 0:cols], scalar1=0.5,
                    scalar2=None, op0=ALU.subtract,
                )
                xts = (x0_t, x1_t)
                rn_ts = [rnp.tile([P, FALLOC], bf16, name=f"rn{i}_{ci}", tag="rn")
                         for i in range(2)]
                # phase 1 (DVE): z'' = s_t * x, in place over x
                for i, xt in enumerate(xts):
                    nc.vector.tensor_tensor(
                        xt[:, 0:cols], s_t[:, 0:cols], xt[:, 0:cols], ALU.mult
                    )
                # phase 2 (ACT): rn = relu(-2 z'') on front ka cols with an
                # exact fused sum; (DVE): same on back cols via 2-op ts.
                for i, xt in enumerate(xts):
                    if ka:
                        nc.scalar.activation(
                            rn_ts[i][:, 0:ka], xt[:, 0:ka], AFT.Relu,
                            scale=-2.0, accum_out=Vc[:, 2 + i : 3 + i],
                        )
                for i, xt in enumerate(xts):
                    nc.vector.tensor_scalar(
                        out=rn_ts[i][:, ka:cols], in0=xt[:, ka:cols],
                        scalar1=-2.0, scalar2=0.0, op0=ALU.mult, op1=ALU.max,
                    )
                # phase 3: a = z'' + rn = |z''| in place; then sigma in place
                for i, xt in enumerate(xts):
                    nc.vector.tensor_tensor(
                        xt[:, 0:cols], xt[:, 0:cols], rn_ts[i][:, 0:cols],
                        ALU.add
                    )
                    nc.scalar.activation(
                        xt[:, 0:cols], xt[:, 0:cols], AFT.Sigmoid,
                        bias=gam_t[:], scale=-2.0 * BETA,
                        accum_out=Vc[:, i : i + 1],
                    )
                # phase 4 (PE): chains over the back rn cols
                for i in range(2):
                    for j in range(ka, cols, TBLK):
                        nc.tensor.matmul(
                            pr[i][:], ones_b[:], rn_ts[i][:, j : j + TBLK],
                            start=(rcols[i] == 0),
                            stop=(rcols[i] + TBLK == rtot),
                        )
                        rcols[i] += TBLK

                # PE chain: per-sample sum(t)
                for j in range(cols // TBLK):
                    off = c0 + j * TBLK
                    nc.tensor.matmul(
                        pt[s][:], ones_b[:], t_t[:, j * TBLK : (j + 1) * TBLK],
                        start=(off == 0), stop=(off + TBLK == FREE_PER_SAMPLE),
                    )

                # fold this chunk's stats into PSUM (fp32r ones-matmul)
                nc.tensor.matmul(
                    U[:], ones_f[:], Vc[:],
                    start=(ci == 0), stop=(ci == n_chunks - 1),
                )

                if ci == N_CHUNKS0 - 1:
                    # sample 0's t-chain just stopped; drain it early
                    nc.vector.tensor_reduce(
                        out=Us[0:1, 8:9], in_=pt[0][:],
                        axis=mybir.AxisListType.X, op=ALU.add,
                    )

            nc.vector.tensor_reduce(
                out=Us[0:1, 9:10], in_=pt[1][:],
                axis=mybir.AxisListType.X, op=ALU.add,
            )
            for i in range(2):
                nc.vector.tensor_reduce(
                    out=Us[0:1, 10 + i : 11 + i], in_=pr[i][:],
                    axis=mybir.AxisListType.X, op=ALU.add,
                )
            nc.vector.tensor_copy(Us[0:1, 0:8], U[0:1, 0:8])
            nc.sync.dma_start(res[0:1, :], Us[:])

    nc.finalize()
    return nc


def _get_nc():
    if "nc" not in _CACHE:
        _CACHE["nc"] = _build()
    return _CACHE["nc"]


def _run(in_maps, trace=False):
    from concourse.bass_utils import run_bass_kernel_spmd

    return run_bass_kernel_spmd(
        _get_nc(), in_maps, core_ids=list(range(N_CORES)), trace=trace
    )


def make_in_maps(out0, out1, out2, targets):
    bf = ml_dtypes.bfloat16
    out0 = np.asarray(out0, dtype=np.float32).astype(bf)
    out1 = np.asarray(out1, dtype=np.float32).astype(bf)
    targets = np.asarray(targets, dtype=np.float32).astype(bf)
    out2 = np.asarray(out2, dtype=np.float32)
    in_maps = []
    for c in range(N_CORES):
        sl = slice(c * B_LOCAL, (c + 1) * B_LOCAL)
        in_maps.append(
            {
                "out0": np.ascontiguousarray(out0[sl]).reshape(ROWS, FREE_PER_SAMPLE),
                "out1": np.ascontiguousarray(out1[sl]).reshape(ROWS, FREE_PER_SAMPLE),
                "targets": np.ascontiguousarray(targets[sl]).reshape(
                    ROWS, FREE_PER_SAMPLE
                ),
                "out2": np.ascontiguousarray(out2[sl]).reshape(1, B_LOCAL * C),
            }
        )
    return in_maps


def combine_partials(stats, out2):
    """Host-side O(1) combine. stats: [N_CORES, 16] per-core sums.
    cols: 0,1=sum sig per tensor; 2,3=ACT-side sum relu(z) per tensor;
    4=sum relu(o2); 5=sum sig(o2); 8,9=sum(t) per sample;
    10,11=PE-side sum relu(z) per tensor."""
    total_main = 0.0
    total_se = 0.0
    for c in range(len(stats)):
        v = [float(x) for x in stats[c]]
        s0 = v[2] + v[10] + ALPHA * v[0] + DELTA * N_CORE_T
        s1 = v[3] + v[11] + ALPHA * v[1] + DELTA * N_CORE_T
        total_main += s0 + AUX_WEIGHT * s1
        total_se += v[4] + ALPHA * v[5] + DELTA * (B_LOCAL * C)
        for s in range(B_LOCAL):
            t_sum = v[8 + s]
            b_global = c * B_LOCAL + s
            if t_sum < ELEMS_PER_SAMPLE - 0.5:  # class-bin 0 present
                total_se -= float(out2[b_global, 0])
            if t_sum > 0.5:  # class-bin 1 present
                total_se -= float(out2[b_global, 1])
    return total_main / N_TOTAL + SE_WEIGHT * total_se / N_SE


def kernel(out0, out1, out2, targets):
    out2 = np.asarray(out2, dtype=np.float32)
    br = _run(make_in_maps(out0, out1, out2, targets))
    stats = [r["stats"][0] for r in br.results]
    return np.asarray(combine_partials(stats, out2), dtype=np.float32)


# revision 15
# speedup vs baseline: 1.3876x; 1.0212x over previous
"""AuxSeLoss v7: bf16, engine-balanced softplus via sigmoid+relu split.

Math: t in {0,1} exactly -> per-element BCE = softplus(z), z = (1-2t)x.
softplus(z) = relu(z) + g(|z|), g(u) = log1p(exp(-u)) ~= A*sigmoid(-B*u+G)+D
(minimax fit, max |err| 4.9e-4).  With z'' = (t-0.5)x (so z = -2z''):
relu(z) = rn := max(-2z'', 0), |z''| = z'' + rn.

Engine balance (HW-measured modes: TT=2x, plain TS=4x, any DVE accum=1x,
ACT=1/cyc with free exact accumulator, PE chain mm ~400ns/512 cols):
  DVE per chunk: s_t = t-0.5 [ts F/4]; per tensor: z''=s_t*x in place
    [tt F/2], rn on the back (1-rho) cols [ts 2-op], a=z''+rn in place
    [tt F/2], sigma in place [ACT].
  ACT: per tensor one full-width Sigmoid pass (scale=-2B, accum) plus a
    Relu pass (scale=-2, accum) on the front rho~1/3 cols - writing the
    same rn tile slice DVE skips; its accumulator supplies that slice's
    sum(relu(z)) exactly.
  PE: ones-chains (512 blocks) for per-sample sum(t) and the back-cols
    sum(rn); per-chunk [P,16] Vc stat fold.
  DMA: t rides the ACT HWDGE queue (nc.scalar), x0/x1 the sync queue, so
    the t tile for chunk c+1 never waits behind x1 of chunk c.
Host: bf16 cast + O(1) combine.  rel err ~1e-4 (gate 2e-2).
"""

import numpy as np
import ml_dtypes

N_CLASSES = 21
B, C, H, W = 16, N_CLASSES, 256, 256
N_CORES = 8
B_LOCAL = B // N_CORES
ELEMS_PER_SAMPLE = C * H * W  # 1376256
P = 128
FREE_PER_SAMPLE = ELEMS_PER_SAMPLE // P  # 10752
ROWS = B_LOCAL * P
AUX_WEIGHT = 0.4
SE_WEIGHT = 0.2
N_TOTAL = B * C * H * W
N_SE = B * C
N_CORE_T = B_LOCAL * ELEMS_PER_SAMPLE  # elems per tensor per core

# minimax fit of log1p(exp(-u)) ~= ALPHA*sigmoid(-BETA*u+GAMMA)+DELTA, u>=0
ALPHA = 2.49059269
BETA = 0.985901754
GAMMA = -0.954224925
DELTA = 1.79845165e-4

# (cols, act_relu_cols) per chunk; act cols ride ACT, rest DVE+PE.
CHUNK_SCHEDULE = [
    [(1024, 0), (2048, 512), (3072, 1024), (4608, 1536)],
    [(5632, 1536), (4608, 1024), (512, 0)],
]
assert all(sum(c for c, _ in cs) == FREE_PER_SAMPLE for cs in CHUNK_SCHEDULE)
assert all(c % 512 == 0 and k % 512 == 0 and k < c
           for cs in CHUNK_SCHEDULE for c, k in cs)
N_CHUNKS0 = len(CHUNK_SCHEDULE[0])
FALLOC = 5776  # 5632 max cols + 288B pad so bank phases differ
TBLK = 512  # PE chain block width (one PSUM bank of f32)

_CACHE: dict = {}


def _build():
    from contextlib import ExitStack

    import concourse.bacc as bacc
    import concourse.mybir as mybir
    from concourse.tile import TileContext

    f32 = mybir.dt.float32
    bf16 = mybir.dt.bfloat16
    AFT = mybir.ActivationFunctionType
    ALU = mybir.AluOpType

    nc = bacc.Bacc("TRN2", target_bir_lowering=False)
    x0 = nc.dram_tensor("out0", [ROWS, FREE_PER_SAMPLE], bf16, kind="ExternalInput")
    x1 = nc.dram_tensor("out1", [ROWS, FREE_PER_SAMPLE], bf16, kind="ExternalInput")
    tg = nc.dram_tensor("targets", [ROWS, FREE_PER_SAMPLE], bf16, kind="ExternalInput")
    o2 = nc.dram_tensor("out2", [1, B_LOCAL * C], f32, kind="ExternalInput")
    res = nc.dram_tensor("stats", [1, 16], f32, kind="ExternalOutput")

    with ExitStack() as ctx, TileContext(nc) as tc:
        with (
            tc.tile_pool(name="x0p", bufs=3) as x0p,
            tc.tile_pool(name="x1p", bufs=3) as x1p,
            tc.tile_pool(name="tp", bufs=3) as tp,
            tc.tile_pool(name="sp", bufs=2) as sp,
            tc.tile_pool(name="rnp", bufs=4) as rnp,
            tc.tile_pool(name="vcp", bufs=2) as vcp,
            tc.tile_pool(name="accp", bufs=1) as accp,
            tc.tile_pool(name="psp", bufs=1, space="PSUM") as psp,
        ):
            ones_f = accp.tile([P, 1], f32)
            nc.vector.memset(ones_f[:], 1.0)
            ones_b = accp.tile([P, 1], bf16)
            nc.vector.memset(ones_b[:], 1.0)
            gam_t = accp.tile([P, 1], f32)
            nc.vector.memset(gam_t[:], GAMMA)
            Us = accp.tile([1, 16], f32)
            o2_t = accp.tile([1, B_LOCAL * C], f32)
            a_o2 = accp.tile([1, B_LOCAL * C], f32)
            r_o2 = accp.tile([1, B_LOCAL * C], f32)
            U = psp.tile([1, 16], f32)
            warm = psp.tile([1, 16], f32)
            pt = [psp.tile([1, TBLK], f32, name=f"pt{s}") for s in range(B_LOCAL)]
            pr = [psp.tile([1, TBLK], f32, name=f"pr{i}") for i in range(2)]

            chunks = []
            for s in range(B_LOCAL):
                c0 = 0
                for cols, ka in CHUNK_SCHEDULE[s]:
                    chunks.append((s, c0, cols, ka))
                    c0 += cols
            n_chunks = len(chunks)
            # total PE rn-chain cols per tensor (back cols of every chunk)
            rtot = sum(c - k for cs in CHUNK_SCHEDULE for c, k in cs)
            rcols = [0, 0]

            # HAM warm-up: ~4.5us of back-to-back tiny matmuls while the
            # DMA pipe fills; flips the PE clock gate to 8/8 (2.4 GHz)
            # before the real ones-chains start.
            for w in range(24):
                nc.tensor.matmul(warm[0:1, 0:1], ones_f[:], gam_t[:],
                                 start=True, stop=True)

            first = True
            for ci, (s, c0, cols, ka) in enumerate(chunks):
                r0, r1 = s * P, (s + 1) * P
                c1 = c0 + cols
                t_t = tp.tile([P, FALLOC], bf16, name=f"t_{ci}", tag="t")
                x0_t = x0p.tile([P, FALLOC], bf16, name=f"x0_{ci}", tag="x0")
                x1_t = x1p.tile([P, FALLOC], bf16, name=f"x1_{ci}", tag="x1")
                nc.sync.dma_start(t_t[:, 0:cols], tg[r0:r1, c0:c1])
                nc.sync.dma_start(x0_t[:, 0:cols], x0[r0:r1, c0:c1])
                nc.sync.dma_start(x1_t[:, 0:cols], x1[r0:r1, c0:c1])

                Vc = vcp.tile([P, 16], f32, name=f"vc_{ci}", tag="vc")
                nc.gpsimd.memset(Vc[:], 0.0)

                if first:
                    # out2 warmup: tiny abs+sigmoid forces the sigmoid
                    # table load before the first bulk ACT op.
                    nc.sync.dma_start(o2_t[:], o2[0:1, :])
                    nc.vector.scalar_tensor_tensor(
                        out=a_o2[:], in0=o2_t[:], scalar=-1.0, in1=o2_t[:],
                        op0=ALU.mult, op1=ALU.max,
                    )
                    nc.vector.tensor_scalar(
                        out=r_o2[:], in0=o2_t[:], scalar1=0.0, scalar2=None,
                        op0=ALU.max, op1=ALU.add,
                        accum_out=Vc[0:1, 4:5],
                    )
                    nc.scalar.activation(
                        a_o2[:], a_o2[:], AFT.Sigmoid,
                        bias=gam_t[0:1], scale=-BETA,
                        accum_out=Vc[0:1, 5:6],
                    )
                    first = False

                s_t = sp.tile([P, FALLOC], bf16, name=f"s_{ci}", tag="s")
                nc.vector.tensor_scalar(
                    out=s_t[:, 0:cols], in0=t_t[:, 0:cols], scalar1=0.5,
                    scalar2=None, op0=ALU.subtract,
                )
                xts = (x0_t, x1_t)
                rn_ts = [rnp.tile([P, FALLOC], bf16, name=f"rn{i}_{ci}", tag="rn")
                         for i in range(2)]
                # phase 1 (DVE): z'' = s_t * x, in place over x
                for i, xt in enumerate(xts):
                    nc.vector.tensor_tensor(
                        xt[:, 0:cols], s_t[:, 0:cols], xt[:, 0:cols], ALU.mult
                    )
                # phase 2 (ACT): rn = relu(-2 z'') on front ka cols with an
                # exact fused sum; (DVE): same on back cols via 2-op ts.
                for i, xt in enumerate(xts):
                    if ka:
                        nc.scalar.activation(
                            rn_ts[i][:, 0:ka], xt[:, 0:ka], AFT.Relu,
                            scale=-2.0, accum_out=Vc[:, 2 + i : 3 + i],
                        )
                for i, xt in enumerate(xts):
                    nc.vector.tensor_scalar(
                        out=rn_ts[i][:, ka:cols], in0=xt[:, ka:cols],
                        scalar1=-2.0, scalar2=0.0, op0=ALU.mult, op1=ALU.max,
                    )
                # phase 3: a = z'' + rn = |z''| in place; then sigma in place
                for i, xt in enumerate(xts):
                    nc.vector.tensor_tensor(
                        xt[:, 0:cols], xt[:, 0:cols], rn_ts[i][:, 0:cols],
                        ALU.add
                    )
                    nc.scalar.activation(
                        xt[:, 0:cols], xt[:, 0:cols], AFT.Sigmoid,
                        bias=gam_t[:], scale=-2.0 * BETA,
                        accum_out=Vc[:, i : i + 1],
                    )
                # phase 4 (PE): chains over the back rn cols
                for i in range(2):
                    for j in range(ka, cols, TBLK):
                        nc.tensor.matmul(
                            pr[i][:], ones_b[:], rn_ts[i][:, j : j + TBLK],
                            start=(rcols[i] == 0),
                            stop=(rcols[i] + TBLK == rtot),
                        )
                        rcols[i] += TBLK

                # PE chain: per-sample sum(t)
                for j in range(cols // TBLK):
                    off = c0 + j * TBLK
                    nc.tensor.matmul(
                        pt[s][:], ones_b[:], t_t[:, j * TBLK : (j + 1) * TBLK],
                        start=(off == 0), stop=(off + TBLK == FREE_PER_SAMPLE),
                    )

                # fold this chunk's stats into PSUM (fp32r ones-matmul)
                nc.tensor.matmul(
                    U[:], ones_f[:], Vc[:],
                    start=(ci == 0), stop=(ci == n_chunks - 1),
                )

                if ci == N_CHUNKS0 - 1:
                    # sample 0's t-chain just stopped; drain it early
                    nc.vector.tensor_reduce(
                        out=Us[0:1, 8:9], in_=pt[0][:],
                        axis=mybir.AxisListType.X, op=ALU.add,
                    )

            nc.vector.tensor_reduce(
                out=Us[0:1, 9:10], in_=pt[1][:],
                axis=mybir.AxisListType.X, op=ALU.add,
            )
            for i in range(2):
                nc.vector.tensor_reduce(
                    out=Us[0:1, 10 + i : 11 + i], in_=pr[i][:],
                    axis=mybir.AxisListType.X, op=ALU.add,
                )
            nc.vector.tensor_copy(Us[0:1, 0:8], U[0:1, 0:8])
            nc.sync.dma_start(res[0:1, :], Us[:])

    nc.finalize()
    return nc


def _get_nc():
    if "nc" not in _CACHE:
        _CACHE["nc"] = _build()
    return _CACHE["nc"]


def _run(in_maps, trace=False):
    from concourse.bass_utils import run_bass_kernel_spmd

    return run_bass_kernel_spmd(
        _get_nc(), in_maps, core_ids=list(range(N_CORES)), trace=trace
    )


def make_in_maps(out0, out1, out2, targets):
    bf = ml_dtypes.bfloat16
    out0 = np.asarray(out0, dtype=np.float32).astype(bf)
    out1 = np.asarray(out1, dtype=np.float32).astype(bf)
    targets = np.asarray(targets, dtype=np.float32).astype(bf)
    out2 = np.asarray(out2, dtype=np.float32)
    in_maps = []
    for c in range(N_CORES):
        sl = slice(c * B_LOCAL, (c + 1) * B_LOCAL)
        in_maps.append(
            {
                "out0": np.ascontiguousarray(out0[sl]).reshape(ROWS, FREE_PER_SAMPLE),
                "out1": np.ascontiguousarray(out1[sl]).reshape(ROWS, FREE_PER_SAMPLE),
                "targets": np.ascontiguousarray(targets[sl]).reshape(
                    ROWS, FREE_PER_SAMPLE
                ),
                "out2": np.ascontiguousarray(out2[sl]).reshape(1, B_LOCAL * C),
            }
        )
    return in_maps


def combine_partials(stats, out2):
    """Host-side O(1) combine. stats: [N_CORES, 16] per-core sums.
    cols: 0,1=sum sig per tensor; 2,3=ACT-side sum relu(z) per tensor;
    4=sum relu(o2); 5=sum sig(o2); 8,9=sum(t) per sample;
    10,11=PE-side sum relu(z) per tensor."""
    total_main = 0.0
    total_se = 0.0
    for c in range(len(stats)):
        v = [float(x) for x in stats[c]]
        s0 = v[2] + v[10] + ALPHA * v[0] + DELTA * N_CORE_T
        s1 = v[3] + v[11] + ALPHA * v[1] + DELTA * N_CORE_T
        total_main += s0 + AUX_WEIGHT * s1
        total_se += v[4] + ALPHA * v[5] + DELTA * (B_LOCAL * C)
        for s in range(B_LOCAL):
            t_sum = v[8 + s]
            b_global = c * B_LOCAL + s
            if t_sum < ELEMS_PER_SAMPLE - 0.5:  # class-bin 0 present
                total_se -= float(out2[b_global, 0])
            if t_sum > 0.5:  # class-bin 1 present
                total_se -= float(out2[b_global, 1])
    return total_main / N_TOTAL + SE_WEIGHT * total_se / N_SE


def kernel(out0, out1, out2, targets):
    out2 = np.asarray(out2, dtype=np.float32)
    br = _run(make_in_maps(out0, out1, out2, targets))
    stats = [r["stats"][0] for r in br.results]
    return np.asarray(combine_partials(stats, out2), dtype=np.float32)
